# revision 9
# baseline (speedup 1.0000x reference)
"""ChebConv (K=4) GNN kernel for 8 Trainium2 NeuronCores — v4.

Strategy (1D node partition, pull-mode, matmul-scatter):
  - Nodes sharded 8 ways (6250/core, padded to 6272 = 49 blocks of 128).
  - Block-major local indexing: local node r -> block bk = r//128, row
    p = r%128. Blocks split into Early (bk < 25, 3200 rows/shard) and
    Late (bk >= 25, 3072 rows/shard) halves, each with its own gather
    table (25600 / 24576 rows, both < 2**15 so int16 indices cover them
    without windowing).
  - States y_k = d^{-1/2} X_k.  Per iteration k = 1..3, two phases:
    phase E gathers the contributions of Early-half sources, phase L of
    Late-half sources.  Per (block, phase): dma_gather (SWDGE) the edge
    slots, then per 128-slot tile one matmul ps += M.T @ gathered
    (M host-precomputed bf16 scatter matrices streamed from DRAM; M
    folds -2*re * cnt * ds2[dst]).  Phase E result is saved to SBUF
    (pre-combined with the recurrence term), phase L completes:
      y1 = 0.5*(psE + psL);  y_k = (psE + psL) - y_{k-2}  (lambda_max=2).
  - Trailing pad slots use idx = -1: the gather ucode trims them at
    runtime, so each core pays Q7 descriptor-generation time only for
    its real edges (the compiled call size covers the worst core).
  - Publication: as soon as all Early blocks of iteration k close, an
    AllGather publishes the Early table for iteration k+1 (overlapping
    the rest of iteration k); the Late AllGather at iteration end
    overlaps the next iteration's Early phase.  Iteration 1 gathers
    directly from host-uploaded y0 tables (no staging copy).
  - Final per block: xt = idsq * [y0|y1|y2|y3]; 2 PE transposes ->
    out = relu(xtT.T @ W + b) -> DMA out (overlaps iteration 3).
"""

import math
import sys

import numpy as np

sys.path.insert(0, "/opt/trn_rl_repo")

import concourse.bacc as bacc  # noqa: E402
import concourse.bass as bass  # noqa: E402
import concourse.mybir as mybir  # noqa: E402
import concourse.tile as tile  # noqa: E402
from concourse.bass_utils import run_bass_kernel_spmd  # noqa: E402

P = 128
N_CORES = 8
F_IN = 64
K_CHEB = 4
F_OUT = 256
FP32 = mybir.dt.float32
BF16 = mybir.dt.bfloat16
I16 = mybir.dt.int16

NB = 49            # blocks per shard
NB_E = 25          # early blocks
NB_L = NB - NB_E   # late blocks
ROWS_E = NB_E * P  # 3200
ROWS_L = NB_L * P  # 3072
CH = 8             # max tiles per gather call (1024 idx ucode limit)


# ---------------------------------------------------------------------------
# host-side graph preprocessing
# ---------------------------------------------------------------------------
def preprocess(signal, src, dst, lambda_max, W, b):
    import ml_dtypes

    n_nodes = signal.shape[0]
    n_shard = (n_nodes + N_CORES - 1) // N_CORES          # 6250
    assert NB * P >= n_shard and (NB - 1) * P < n_shard

    deg = np.bincount(dst, minlength=n_nodes).astype(np.float64)
    degc = np.maximum(deg, 1.0)
    dsqrt = (degc ** -0.5).astype(np.float32)
    ds2 = (1.0 / degc).astype(np.float32)
    idsq = (degc ** 0.5).astype(np.float32)

    re = 2.0 / float(np.asarray(lambda_max).reshape(-1)[0])
    c1 = re - 1.0
    assert abs(c1) < 1e-12, "general lambda_max not wired (needs c1 terms)"
    scale1 = np.float32(-2.0 * re)

    # dedup (dst, src) -> cnt
    key = dst.astype(np.int64) * n_nodes + src.astype(np.int64)
    ukey, cnt = np.unique(key, return_counts=True)
    udst = (ukey // n_nodes).astype(np.int64)
    usrc = (ukey % n_nodes).astype(np.int64)

    owner = udst // n_shard
    rloc = udst - owner * n_shard
    bk = rloc // P
    dr = rloc - bk * P

    sc = usrc // n_shard
    rs = usrc - sc * n_shard
    ph = (rs >= ROWS_E).astype(np.int64)
    trow = np.where(ph == 0, sc * ROWS_E + rs, sc * ROWS_L + (rs - ROWS_E))

    w_edge = (scale1 * cnt.astype(np.float32) * ds2[udst]).astype(np.float32)

    # slot = (owner, bk, ph, src); dedup via unique on composite key
    okey = (((owner * NB + bk) * 2 + ph) * n_nodes + usrc)
    slots_key, slot_of_edge = np.unique(okey, return_inverse=True)
    n_slots = len(slots_key)
    s_owner = slots_key // (2 * n_nodes * NB)
    rem = slots_key % (2 * n_nodes * NB)
    s_bk = rem // (2 * n_nodes)
    rem = rem % (2 * n_nodes)
    s_ph = rem // n_nodes
    s_src = rem % n_nodes
    s_sc = s_src // n_shard
    s_rs = s_src - s_sc * n_shard
    s_trow = np.where(s_ph == 0, s_sc * ROWS_E + s_rs,
                      s_sc * ROWS_L + (s_rs - ROWS_E))

    # per (owner, bk, ph) group: count + start (slots_key sorted => groups
    # contiguous in (owner, bk, ph) order)
    gid = (s_owner * NB + s_bk) * 2 + s_ph
    n_groups = N_CORES * NB * 2
    gcnt = np.bincount(gid, minlength=n_groups).reshape(N_CORES, NB, 2)
    gstart = np.zeros(n_groups + 1, dtype=np.int64)
    np.cumsum(gcnt.reshape(-1), out=gstart[1:])
    slot_local = np.arange(n_slots) - gstart[gid]

    # compiled tiles per (bk, ph): cover worst core
    gmax = gcnt.max(axis=0)                               # [NB, 2]
    tiles_bp = np.maximum(1, np.ceil(gmax / P).astype(np.int64))

    # stream order: ph-major, then block; per (bk, ph) calls of <= CH tiles
    tile_off = np.zeros((NB, 2), dtype=np.int64)
    calls = []      # (ph, bk, tile_start, n_tiles, first_of_block, last)
    toff = 0
    for phx in range(2):
        for b_ in range(NB):
            t = int(tiles_bp[b_, phx])
            tile_off[b_, phx] = toff
            s = 0
            while s < t:
                e = min(s + CH, t)
                calls.append((phx, b_, toff + s, e - s, s == 0, e == t))
                s = e
            toff += t
    total_tiles = toff

    # per-core idx + M blob
    in_maps = []
    for c in range(N_CORES):
        idx16 = np.zeros((total_tiles, P), dtype=np.int16)
        mflat = np.zeros(P * total_tiles * P, dtype=np.float32)
        mask = s_owner == c
        sl = slot_local[mask]
        tr = s_trow[mask]
        gb = s_bk[mask]
        gp = s_ph[mask]
        gtile = tile_off[gb, gp] + sl // P
        idx16[gtile, sl % P] = tr.astype(np.int16)
        # M[slot%P (partition), gtile*P + dr (free)]
        emask = owner == c
        es = slot_of_edge[emask]
        # es indexes into global slot arrays; recompute per-edge placement
        e_sl = slot_local[es]
        e_tile = tile_off[s_bk[es], s_ph[es]] + e_sl // P
        e_part = e_sl % P
        e_dr = dr[emask]
        mflat[(e_part * total_tiles + e_tile) * P + e_dr] = w_edge[emask]
        mblob = mflat.reshape(P, total_tiles * P)
        wrap = idx16.reshape(-1, 16).T.copy()             # [16, tt*8]
        in_maps.append({
            "idx": np.tile(wrap, (8, 1)),                 # [128, tt*8]
            "mblob": mblob.astype(ml_dtypes.bfloat16),
        })

    # per-core aux arrays (block-major local layout: node r = bk*128+p)
    def cols_of(vec, fill):
        out = np.full((N_CORES, NB * P), fill, dtype=np.float32)
        out[:, :n_shard] = vec.reshape(N_CORES, n_shard)
        return out.reshape(N_CORES, NB, P).transpose(0, 2, 1).copy()

    idsq_cols = cols_of(idsq, 1.0)

    y0 = np.asarray(signal, np.float32) * dsqrt[:, None]
    y0_pad = np.zeros((N_CORES, NB * P, F_IN), dtype=np.float32)
    y0_pad[:, :n_shard] = y0.reshape(N_CORES, n_shard, F_IN)
    # y0 tables (global upload, identical on all cores)
    t0E = y0_pad[:, :ROWS_E].reshape(N_CORES * ROWS_E, F_IN).copy()
    t0L = y0_pad[:, ROWS_E:].reshape(N_CORES * ROWS_L, F_IN).copy()
    # y0 in SBUF layout [128, nb*64] (partition p, col bk*64+f)
    y0_sb = y0_pad.reshape(N_CORES, NB, P, F_IN).transpose(0, 2, 1, 3) \
        .reshape(N_CORES, P, NB * F_IN).copy()

    w_in = np.asarray(W, np.float32)
    b_rep = np.broadcast_to(np.asarray(b, np.float32), (P, F_OUT)).copy()
    ident = np.eye(P, dtype=np.float32)

    for c in range(N_CORES):
        in_maps[c].update({
            "t0E": t0E, "t0L": t0L,
            "y0sb": y0_sb[c].copy(),
            "idsq": idsq_cols[c],
            "w_in": w_in, "b_rep": b_rep, "ident": ident,
        })

    cfg = dict(
        n_nodes=n_nodes, n_shard=n_shard,
        total_tiles=total_tiles, calls=tuple(calls),
    )
    return cfg, in_maps


# ---------------------------------------------------------------------------
# Bass program
# ---------------------------------------------------------------------------
def build_program(cfg):
    total_tiles = cfg["total_tiles"]
    calls = cfg["calls"]

    nc = bacc.Bacc(
        "TRN2", target_bir_lowering=False, debug=False,
        enable_asserts=False, num_devices=N_CORES,
    )

    t0E_d = nc.dram_tensor("t0E", [N_CORES * ROWS_E, F_IN], FP32,
                           kind="ExternalInput")
    t0L_d = nc.dram_tensor("t0L", [N_CORES * ROWS_L, F_IN], FP32,
                           kind="ExternalInput")
    idx_d = nc.dram_tensor("idx", [P, total_tiles * 8], I16,
                           kind="ExternalInput")
    m_d = nc.dram_tensor("mblob", [P, total_tiles * P], BF16,
                         kind="ExternalInput")
    y0sb_d = nc.dram_tensor("y0sb", [P, NB * F_IN], FP32,
                            kind="ExternalInput")
    idsq_d = nc.dram_tensor("idsq", [P, NB], FP32, kind="ExternalInput")
    w_d = nc.dram_tensor("w_in", [2 * P, F_OUT], FP32, kind="ExternalInput")
    brep_d = nc.dram_tensor("b_rep", [P, F_OUT], FP32, kind="ExternalInput")
    ident_d = nc.dram_tensor("ident", [P, P], FP32, kind="ExternalInput")
    out_d = nc.dram_tensor("out", [NB * P, F_OUT], FP32,
                           kind="ExternalOutput")

    rg = [list(range(N_CORES))]
    mult = mybir.AluOpType.mult
    add = mybir.AluOpType.add
    sub = mybir.AluOpType.subtract
    Relu = mybir.ActivationFunctionType.Relu

    with tile.TileContext(nc) as tc:
        with (
            tc.tile_pool(name="const", bufs=1) as constp,
            tc.tile_pool(name="state", bufs=1) as statep,
            tc.tile_pool(name="chunk", bufs=3) as chunkp,
            tc.tile_pool(name="mchunk", bufs=3) as mchp,
            tc.tile_pool(name="work", bufs=4) as workp,
            tc.tile_pool(name="psA", bufs=3, space="PSUM") as psp,
            tc.tile_pool(name="psT", bufs=2, space="PSUM") as pstp,
            tc.tile_pool(name="psO", bufs=1, space="PSUM") as psop,
            tc.tile_pool(name="dram", bufs=2, space="DRAM") as dramp,
        ):
            # staged y0 tables (SWDGE gather needs compile-time addresses,
            # so inputs must be copied into internal DRAM first)
            tabE0 = dramp.tile([N_CORES * ROWS_E, F_IN], FP32,
                               tag="tabE0", name="tabE0", bufs=1)
            nc.sync.dma_start(tabE0[:], t0E_d[:])
            tabL0 = dramp.tile([N_CORES * ROWS_L, F_IN], FP32,
                               tag="tabL0", name="tabL0", bufs=1)
            nc.sync.dma_start(tabL0[:], t0L_d[:])

            # publish buffers + gather tables for iterations 2, 3
            agE_in = {}
            agL_in = {}
            tabE = {}
            tabL = {}
            for k in (1, 2):
                agE_in[k] = dramp.tile([ROWS_E, F_IN], FP32, tag="agE",
                                       name=f"agE{k}", bufs=2)
                agL_in[k] = dramp.tile([ROWS_L, F_IN], FP32, tag="agL",
                                       name=f"agL{k}", bufs=2)
                tabE[k] = dramp.tile([N_CORES * ROWS_E, F_IN], FP32,
                                     tag="tabE", name=f"tabE{k}", bufs=2)
                tabL[k] = dramp.tile([N_CORES * ROWS_L, F_IN], FP32,
                                     tag="tabL", name=f"tabL{k}", bufs=2)
            # ---- constants
            idx_t = constp.tile([P, total_tiles * 8], I16, tag="idx")
            nc.sync.dma_start(idx_t[:], idx_d[:])
            idsq_t = constp.tile([P, NB], FP32, tag="idsq")
            nc.sync.dma_start(idsq_t[:], idsq_d[:])
            w1_t = constp.tile([P, F_OUT], FP32, tag="w1")
            nc.sync.dma_start(w1_t[:], w_d[0:P, :])
            w2_t = constp.tile([P, F_OUT], FP32, tag="w2")
            nc.sync.dma_start(w2_t[:], w_d[P:2 * P, :])
            brep_t = constp.tile([P, F_OUT], FP32, tag="brep")
            nc.sync.dma_start(brep_t[:], brep_d[:])
            ident_t = constp.tile([P, P], FP32, tag="ident")
            nc.sync.dma_start(ident_t[:], ident_d[:])
            zero_t = constp.tile([P, 512], FP32, tag="zero")
            nc.vector.memset(zero_t[:], 0.0)

            # ---- states
            ybuf = statep.tile([P, NB * 4 * F_IN], FP32, tag="ybuf")
            for b_ in range(NB):
                nc.sync.dma_start(
                    ybuf[:, b_ * 256:b_ * 256 + F_IN],
                    y0sb_d[:, b_ * F_IN:(b_ + 1) * F_IN],
                )
            esb = statep.tile([P, NB * F_IN], FP32, tag="esb")

            def ysl(b_, k):
                o = b_ * 256 + k * F_IN
                return ybuf[:, o:o + F_IN]

            def esl(b_):
                return esb[:, b_ * F_IN:(b_ + 1) * F_IN]

            # prime the gather-destination buffers (pads are skipped by the
            # ucode and would otherwise expose uninitialized SBUF -> NaN*0)
            for r_ in range(3):
                ctp = chunkp.tile([P, CH, F_IN], FP32, tag="ct",
                                  name=f"prime{r_}", bufs=3)
                nc.vector.memset(ctp[:], 0.0)

            def final_block(b_):
                xt = workp.tile([P, 4 * F_IN], FP32, tag="xt")
                nc.vector.tensor_scalar(
                    out=xt[:], in0=ybuf[:, b_ * 256:(b_ + 1) * 256],
                    scalar1=idsq_t[:, b_:b_ + 1], scalar2=None, op0=mult,
                )
                pso = psop.tile([P, F_OUT], FP32, tag="po")
                for h in range(2):
                    pst = pstp.tile([P, P], FP32, tag="tp")
                    nc.tensor.transpose(
                        pst[:], xt[:, h * P:(h + 1) * P], ident_t[:]
                    )
                    xtT = workp.tile([P, P], FP32, tag="xtT")
                    nc.vector.tensor_copy(out=xtT[:], in_=pst[:])
                    nc.tensor.matmul(
                        out=pso[:], lhsT=xtT[:],
                        rhs=(w1_t[:] if h == 0 else w2_t[:]),
                        start=(h == 0), stop=(h == 1),
                    )
                v = workp.tile([P, F_OUT], FP32, tag="fo")
                nc.vector.tensor_tensor(
                    out=v[:], in0=pso[:], in1=brep_t[:], op=add
                )
                r_ = workp.tile([P, F_OUT], FP32, tag="fo2")
                nc.scalar.activation(r_[:], v[:], Relu)
                nc.sync.dma_start(out_d[b_ * P:(b_ + 1) * P, :], r_[:])

            for k in range(1, K_CHEB):
                acc = {}          # (ph, group) -> psum tile

                def ps_sl(b_, phx, k=k, acc=acc):
                    g = b_ // 8
                    return acc[(phx, g)][:, (b_ % 8) * F_IN:(b_ % 8 + 1) * F_IN]

                for (phx, b_, ts, ntl, first, last) in calls:
                    if phx == 0:
                        table = tabE0 if k == 1 else tabE[k - 1]
                    else:
                        table = tabL0 if k == 1 else tabL[k - 1]
                    ct = chunkp.tile([P, CH, F_IN], FP32, tag="ct",
                                     name=f"ct{k}_{ts}", bufs=3)
                    nc.gpsimd.dma_gather(
                        ct[:, 0:ntl, :], table[:],
                        idx_t[:, ts * 8:(ts + ntl) * 8],
                        ntl * P, ntl * P, F_IN,
                    )
                    mt = mchp.tile([P, CH * P], BF16, tag="mt",
                                   name=f"mt{k}_{ts}", bufs=3)
                    nc.sync.dma_start(
                        mt[:, 0:ntl * P], m_d[:, ts * P:(ts + ntl) * P]
                    )
                    ctb = chunkp.tile([P, CH, F_IN], BF16, tag="ctb",
                                      name=f"ctb{k}_{ts}", bufs=3)
                    nc.vector.tensor_copy(out=ctb[:, 0:ntl, :],
                                          in_=ct[:, 0:ntl, :])
                    g = b_ // 8
                    if (phx, g) not in acc:
                        acc[(phx, g)] = psp.tile(
                            [P, 512], FP32, tag="acc",
                            name=f"acc{k}_{phx}_{g}", bufs=3,
                        )
                        nc.tensor.matmul(
                            out=acc[(phx, g)][:],
                            lhsT=zero_t[:, 0:P], rhs=zero_t[:],
                            start=True, stop=False,
                            skip_group_check=True,
                        )
                    for j in range(ntl):
                        nc.tensor.matmul(
                            out=ps_sl(b_, phx),
                            lhsT=mt[:, j * P:(j + 1) * P],
                            rhs=ctb[:, j, :],
                            start=False, stop=(last and j == ntl - 1),
                            skip_group_check=True,
                        )
                    if not last:
                        continue
                    # ---- block close for this phase
                    if phx == 0:
                        if k == 1:
                            nc.vector.tensor_scalar(
                                out=esl(b_), in0=ps_sl(b_, 0),
                                scalar1=0.5, scalar2=None, op0=mult,
                            )
                        else:
                            nc.vector.tensor_tensor(
                                out=esl(b_), in0=ps_sl(b_, 0),
                                in1=ysl(b_, k - 2), op=sub,
                            )
                    else:
                        if k == 1:
                            nc.vector.scalar_tensor_tensor(
                                out=ysl(b_, 1), in0=ps_sl(b_, 1),
                                scalar=0.5, in1=esl(b_),
                                op0=mult, op1=add,
                            )
                        else:
                            nc.vector.tensor_tensor(
                                out=ysl(b_, k), in0=ps_sl(b_, 1),
                                in1=esl(b_), op=add,
                            )
                        if k < K_CHEB - 1:
                            if b_ < NB_E:
                                nc.sync.dma_start(
                                    agE_in[k][b_ * P:(b_ + 1) * P, :],
                                    ysl(b_, k),
                                )
                            else:
                                bl = b_ - NB_E
                                nc.sync.dma_start(
                                    agL_in[k][bl * P:(bl + 1) * P, :],
                                    ysl(b_, k),
                                )
                            if b_ == NB_E - 1:
                                nc.gpsimd.collective_compute(
                                    "AllGather", mybir.AluOpType.bypass,
                                    replica_groups=rg,
                                    ins=[agE_in[k][:].opt()],
                                    outs=[tabE[k][:].opt()],
                                )
                            if b_ == NB - 1:
                                nc.gpsimd.collective_compute(
                                    "AllGather", mybir.AluOpType.bypass,
                                    replica_groups=rg,
                                    ins=[agL_in[k][:].opt()],
                                    outs=[tabL[k][:].opt()],
                                )
                        else:
                            final_block(b_)

    nc.compile()
    return nc


# ---------------------------------------------------------------------------
# entry point
# ---------------------------------------------------------------------------
_CACHE = {}


def _run(signal, src, dst, lambda_max, W, b, trace=False):
    cfg, in_maps = preprocess(signal, src, dst, lambda_max, W, b)
    key = (cfg["total_tiles"], cfg["calls"])
    if key not in _CACHE:
        _CACHE[key] = build_program(cfg)
    nc = _CACHE[key]
    res = run_bass_kernel_spmd(
        nc, in_maps, core_ids=list(range(N_CORES)), trace=trace
    )
    n_shard = cfg["n_shard"]
    outs = []
    for c in range(N_CORES):
        o = res.results[c]["out"]                      # [6272, 256]
        outs.append(o[:n_shard])
    full = np.concatenate(outs, axis=0)[:cfg["n_nodes"]]
    return full, res


def kernel(signal, src, dst, lambda_max, W, b):
    signal = np.asarray(signal, np.float32)
    src = np.asarray(src, np.int32)
    dst = np.asarray(dst, np.int32)
    lambda_max = np.asarray(lambda_max, np.float32)
    W = np.asarray(W, np.float32)
    b = np.asarray(b, np.float32)
    out, _ = _run(signal, src, dst, lambda_max, W, b, trace=False)
    return out


# revision 15
# speedup vs baseline: 1.0441x; 1.0441x over previous
"""ChebConv (K=4) GNN kernel for 8 Trainium2 NeuronCores — v4.

Strategy (1D node partition, pull-mode, matmul-scatter):
  - Nodes sharded 8 ways (6250/core, padded to 6272 = 49 blocks of 128).
  - Block-major local indexing: local node r -> block bk = r//128, row
    p = r%128. Blocks split into Early (bk < 25, 3200 rows/shard) and
    Late (bk >= 25, 3072 rows/shard) halves, each with its own gather
    table (25600 / 24576 rows, both < 2**15 so int16 indices cover them
    without windowing).
  - States y_k = d^{-1/2} X_k.  Per iteration k = 1..3, two phases:
    phase E gathers the contributions of Early-half sources, phase L of
    Late-half sources.  Per (block, phase): dma_gather (SWDGE) the edge
    slots, then per 128-slot tile one matmul ps += M.T @ gathered
    (M host-precomputed bf16 scatter matrices streamed from DRAM; M
    folds -2*re * cnt * ds2[dst]).  Phase E result is saved to SBUF
    (pre-combined with the recurrence term), phase L completes:
      y1 = 0.5*(psE + psL);  y_k = (psE + psL) - y_{k-2}  (lambda_max=2).
  - Trailing pad slots use idx = -1: the gather ucode trims them at
    runtime, so each core pays Q7 descriptor-generation time only for
    its real edges (the compiled call size covers the worst core).
  - Publication: as soon as all Early blocks of iteration k close, an
    AllGather publishes the Early table for iteration k+1 (overlapping
    the rest of iteration k); the Late AllGather at iteration end
    overlaps the next iteration's Early phase.  Iteration 1 gathers
    directly from host-uploaded y0 tables (no staging copy).
  - Final per block: xt = idsq * [y0|y1|y2|y3]; 2 PE transposes ->
    out = relu(xtT.T @ W + b) -> DMA out (overlaps iteration 3).
"""

import math
import sys

import numpy as np

sys.path.insert(0, "/opt/trn_rl_repo")

import concourse.bacc as bacc  # noqa: E402
import concourse.bass as bass  # noqa: E402
import concourse.mybir as mybir  # noqa: E402
import concourse.tile as tile  # noqa: E402
from concourse.bass_utils import run_bass_kernel_spmd  # noqa: E402

P = 128
N_CORES = 8
F_IN = 64
K_CHEB = 4
F_OUT = 256
FP32 = mybir.dt.float32
BF16 = mybir.dt.bfloat16
I16 = mybir.dt.int16

NB = 49            # blocks per shard
NB_E = 25          # early blocks
NB_L = NB - NB_E   # late blocks
ROWS_E = NB_E * P  # 3200
ROWS_L = NB_L * P  # 3072
CH = 8             # max tiles per gather call (1024 idx ucode limit)


# ---------------------------------------------------------------------------
# host-side graph preprocessing
# ---------------------------------------------------------------------------
def preprocess(signal, src, dst, lambda_max, W, b):
    import ml_dtypes

    n_nodes = signal.shape[0]
    n_shard = (n_nodes + N_CORES - 1) // N_CORES          # 6250
    assert NB * P >= n_shard and (NB - 1) * P < n_shard

    deg = np.bincount(dst, minlength=n_nodes).astype(np.float64)
    degc = np.maximum(deg, 1.0)
    dsqrt = (degc ** -0.5).astype(np.float32)
    ds2 = (1.0 / degc).astype(np.float32)
    idsq = (degc ** 0.5).astype(np.float32)

    re = 2.0 / float(np.asarray(lambda_max).reshape(-1)[0])
    c1 = re - 1.0
    assert abs(c1) < 1e-12, "general lambda_max not wired (needs c1 terms)"
    scale1 = np.float32(-2.0 * re)

    # dedup (dst, src) -> cnt
    key = dst.astype(np.int64) * n_nodes + src.astype(np.int64)
    ukey, cnt = np.unique(key, return_counts=True)
    udst = (ukey // n_nodes).astype(np.int64)
    usrc = (ukey % n_nodes).astype(np.int64)

    owner = udst // n_shard
    rloc = udst - owner * n_shard
    bk = rloc // P
    dr = rloc - bk * P

    sc = usrc // n_shard
    rs = usrc - sc * n_shard
    ph = (rs >= ROWS_E).astype(np.int64)
    trow = np.where(ph == 0, sc * ROWS_E + rs, sc * ROWS_L + (rs - ROWS_E))

    w_edge = (scale1 * cnt.astype(np.float32) * ds2[udst]).astype(np.float32)

    # slot = (owner, bk, ph, src); dedup via unique on composite key
    okey = (((owner * NB + bk) * 2 + ph) * n_nodes + usrc)
    slots_key, slot_of_edge = np.unique(okey, return_inverse=True)
    n_slots = len(slots_key)
    s_owner = slots_key // (2 * n_nodes * NB)
    rem = slots_key % (2 * n_nodes * NB)
    s_bk = rem // (2 * n_nodes)
    rem = rem % (2 * n_nodes)
    s_ph = rem // n_nodes
    s_src = rem % n_nodes
    s_sc = s_src // n_shard
    s_rs = s_src - s_sc * n_shard
    s_trow = np.where(s_ph == 0, s_sc * ROWS_E + s_rs,
                      s_sc * ROWS_L + (s_rs - ROWS_E))

    # per (owner, bk, ph) group: count + start (slots_key sorted => groups
    # contiguous in (owner, bk, ph) order)
    gid = (s_owner * NB + s_bk) * 2 + s_ph
    n_groups = N_CORES * NB * 2
    gcnt = np.bincount(gid, minlength=n_groups).reshape(N_CORES, NB, 2)
    gstart = np.zeros(n_groups + 1, dtype=np.int64)
    np.cumsum(gcnt.reshape(-1), out=gstart[1:])
    slot_local = np.arange(n_slots) - gstart[gid]

    # compiled tiles per (bk, ph): cover worst core
    gmax = gcnt.max(axis=0)                               # [NB, 2]
    tiles_bp = np.maximum(1, np.ceil(gmax / P).astype(np.int64))

    # stream order: ph-major, then block; tiles_seq = (ph, bk, last_of_block)
    tile_off = np.zeros((NB, 2), dtype=np.int64)
    tiles_seq = []
    toff = 0
    for phx in range(2):
        for b_ in range(NB):
            t = int(tiles_bp[b_, phx])
            tile_off[b_, phx] = toff
            for j in range(t):
                tiles_seq.append((phx, b_, j == t - 1))
            toff += t
    total_tiles = toff

    # per-core idx + M blob
    in_maps = []
    for c in range(N_CORES):
        idx16 = np.zeros((total_tiles, P), dtype=np.int16)
        mflat = np.zeros(P * total_tiles * P, dtype=np.float32)
        mask = s_owner == c
        sl = slot_local[mask]
        tr = s_trow[mask]
        gb = s_bk[mask]
        gp = s_ph[mask]
        gtile = tile_off[gb, gp] + sl // P
        idx16[gtile, sl % P] = tr.astype(np.int16)
        # M[slot%P (partition), gtile*P + dr (free)]
        emask = owner == c
        es = slot_of_edge[emask]
        # es indexes into global slot arrays; recompute per-edge placement
        e_sl = slot_local[es]
        e_tile = tile_off[s_bk[es], s_ph[es]] + e_sl // P
        e_part = e_sl % P
        e_dr = dr[emask]
        mflat[(e_part * total_tiles + e_tile) * P + e_dr] = w_edge[emask]
        mblob = mflat.reshape(P, total_tiles * P)
        wrap = idx16.reshape(-1, 16).T.copy()             # [16, tt*8]
        in_maps.append({
            "idx": np.tile(wrap, (8, 1)),                 # [128, tt*8]
            "mblob": mblob.astype(ml_dtypes.bfloat16),
        })

    # per-core aux arrays (block-major local layout: node r = bk*128+p)
    def cols_of(vec, fill):
        out = np.full((N_CORES, NB * P), fill, dtype=np.float32)
        out[:, :n_shard] = vec.reshape(N_CORES, n_shard)
        return out.reshape(N_CORES, NB, P).transpose(0, 2, 1).copy()

    idsq_cols = cols_of(idsq, 1.0)

    y0 = np.asarray(signal, np.float32) * dsqrt[:, None]
    y0_pad = np.zeros((N_CORES, NB * P, F_IN), dtype=np.float32)
    y0_pad[:, :n_shard] = y0.reshape(N_CORES, n_shard, F_IN)
    # y0 tables (global upload, identical on all cores)
    t0E = y0_pad[:, :ROWS_E].reshape(N_CORES * ROWS_E, F_IN).copy()
    t0L = y0_pad[:, ROWS_E:].reshape(N_CORES * ROWS_L, F_IN).copy()
    # y0 in SBUF layout [128, nb*64] (partition p, col bk*64+f)
    y0_sb = y0_pad.reshape(N_CORES, NB, P, F_IN).transpose(0, 2, 1, 3) \
        .reshape(N_CORES, P, NB * F_IN).copy()

    w_in = np.asarray(W, np.float32)
    b_rep = np.broadcast_to(np.asarray(b, np.float32), (P, F_OUT)).copy()
    ident = np.eye(P, dtype=np.float32)

    for c in range(N_CORES):
        in_maps[c].update({
            "t0E": t0E, "t0L": t0L,
            "y0sb": y0_sb[c].copy(),
            "idsq": idsq_cols[c],
            "w_in": w_in, "b_rep": b_rep, "ident": ident,
        })

    cfg = dict(
        n_nodes=n_nodes, n_shard=n_shard,
        total_tiles=total_tiles, tiles_seq=tuple(tiles_seq),
    )
    return cfg, in_maps


# ---------------------------------------------------------------------------
# Bass program
# ---------------------------------------------------------------------------
def build_program(cfg):
    total_tiles = cfg["total_tiles"]
    tiles_seq = cfg["tiles_seq"]

    # uniform CH-tile chunks, not crossing phase boundaries
    bounds = [0]
    for i in range(1, total_tiles):
        if tiles_seq[i][0] != tiles_seq[i - 1][0]:
            bounds.append(i)
    bounds.append(total_tiles)
    chunks = []
    for bi in range(len(bounds) - 1):
        s = bounds[bi]
        while s < bounds[bi + 1]:
            e = min(s + CH, bounds[bi + 1])
            chunks.append((s, e))
            s = e

    nc = bacc.Bacc(
        "TRN2", target_bir_lowering=False, debug=False,
        enable_asserts=False, num_devices=N_CORES,
    )

    t0E_d = nc.dram_tensor("t0E", [N_CORES * ROWS_E, F_IN], FP32,
                           kind="ExternalInput")
    t0L_d = nc.dram_tensor("t0L", [N_CORES * ROWS_L, F_IN], FP32,
                           kind="ExternalInput")
    idx_d = nc.dram_tensor("idx", [P, total_tiles * 8], I16,
                           kind="ExternalInput")
    m_d = nc.dram_tensor("mblob", [P, total_tiles * P], BF16,
                         kind="ExternalInput")
    y0sb_d = nc.dram_tensor("y0sb", [P, NB * F_IN], FP32,
                            kind="ExternalInput")
    idsq_d = nc.dram_tensor("idsq", [P, NB], FP32, kind="ExternalInput")
    w_d = nc.dram_tensor("w_in", [2 * P, F_OUT], FP32, kind="ExternalInput")
    brep_d = nc.dram_tensor("b_rep", [P, F_OUT], FP32, kind="ExternalInput")
    ident_d = nc.dram_tensor("ident", [P, P], FP32, kind="ExternalInput")
    out_d = nc.dram_tensor("out", [NB * P, F_OUT], FP32,
                           kind="ExternalOutput")

    rg = [list(range(N_CORES))]
    mult = mybir.AluOpType.mult
    add = mybir.AluOpType.add
    sub = mybir.AluOpType.subtract
    Relu = mybir.ActivationFunctionType.Relu

    with tile.TileContext(nc) as tc:
        with (
            tc.tile_pool(name="const", bufs=1) as constp,
            tc.tile_pool(name="state", bufs=1) as statep,
            tc.tile_pool(name="chunk", bufs=3) as chunkp,
            tc.tile_pool(name="mchunk", bufs=3) as mchp,
            tc.tile_pool(name="work", bufs=4) as workp,
            tc.tile_pool(name="psA", bufs=3, space="PSUM") as psp,
            tc.tile_pool(name="psT", bufs=2, space="PSUM") as pstp,
            tc.tile_pool(name="psO", bufs=1, space="PSUM") as psop,
            tc.tile_pool(name="dram", bufs=2, space="DRAM") as dramp,
        ):
            # staged y0 tables (SWDGE gather needs compile-time addresses,
            # so inputs must be copied into internal DRAM first)
            tabE0 = dramp.tile([N_CORES * ROWS_E, F_IN], FP32,
                               tag="tabE0", name="tabE0", bufs=1)
            nc.sync.dma_start(tabE0[:], t0E_d[:])
            tabL0 = dramp.tile([N_CORES * ROWS_L, F_IN], FP32,
                               tag="tabL0", name="tabL0", bufs=1)
            nc.sync.dma_start(tabL0[:], t0L_d[:])

            # publish buffers + gather tables for iterations 2, 3
            agE_in = {}
            agL_in = {}
            tabE = {}
            tabL = {}
            for k in (1, 2):
                agE_in[k] = dramp.tile([ROWS_E, F_IN], FP32, tag="agE",
                                       name=f"agE{k}", bufs=2)
                agL_in[k] = dramp.tile([ROWS_L, F_IN], FP32, tag="agL",
                                       name=f"agL{k}", bufs=2)
                tabE[k] = dramp.tile([N_CORES * ROWS_E, F_IN], FP32,
                                     tag="tabE", name=f"tabE{k}", bufs=2)
                tabL[k] = dramp.tile([N_CORES * ROWS_L, F_IN], FP32,
                                     tag="tabL", name=f"tabL{k}", bufs=2)
            # ---- constants
            idx_t = constp.tile([P, total_tiles * 8], I16, tag="idx")
            nc.sync.dma_start(idx_t[:], idx_d[:])
            idsq_t = constp.tile([P, NB], FP32, tag="idsq")
            nc.sync.dma_start(idsq_t[:], idsq_d[:])
            w1_t = constp.tile([P, F_OUT], FP32, tag="w1")
            nc.sync.dma_start(w1_t[:], w_d[0:P, :])
            w2_t = constp.tile([P, F_OUT], FP32, tag="w2")
            nc.sync.dma_start(w2_t[:], w_d[P:2 * P, :])
            brep_t = constp.tile([P, F_OUT], FP32, tag="brep")
            nc.sync.dma_start(brep_t[:], brep_d[:])
            ident_t = constp.tile([P, P], FP32, tag="ident")
            nc.sync.dma_start(ident_t[:], ident_d[:])
            zero_t = constp.tile([P, 512], FP32, tag="zero")
            nc.vector.memset(zero_t[:], 0.0)

            # ---- states
            ybuf = statep.tile([P, NB * 4 * F_IN], FP32, tag="ybuf")
            for b_ in range(NB):
                nc.sync.dma_start(
                    ybuf[:, b_ * 256:b_ * 256 + F_IN],
                    y0sb_d[:, b_ * F_IN:(b_ + 1) * F_IN],
                )
            esb = statep.tile([P, NB * F_IN], FP32, tag="esb")

            def ysl(b_, k):
                o = b_ * 256 + k * F_IN
                return ybuf[:, o:o + F_IN]

            def esl(b_):
                return esb[:, b_ * F_IN:(b_ + 1) * F_IN]

            # prime the gather-destination buffers (pads are skipped by the
            # ucode and would otherwise expose uninitialized SBUF -> NaN*0)
            for r_ in range(4):
                ctp = chunkp.tile([P, CH, F_IN], FP32, tag="ct",
                                  name=f"prime{r_}", bufs=4)
                nc.vector.memset(ctp[:], 0.0)

            def final_block(b_):
                xt = workp.tile([P, 4 * F_IN], FP32, tag="xt")
                nc.vector.tensor_scalar(
                    out=xt[:], in0=ybuf[:, b_ * 256:(b_ + 1) * 256],
                    scalar1=idsq_t[:, b_:b_ + 1], scalar2=None, op0=mult,
                )
                pso = psop.tile([P, F_OUT], FP32, tag="po")
                for h in range(2):
                    pst = pstp.tile([P, P], FP32, tag="tp")
                    nc.tensor.transpose(
                        pst[:], xt[:, h * P:(h + 1) * P], ident_t[:]
                    )
                    xtT = workp.tile([P, P], FP32, tag="xtT")
                    nc.vector.tensor_copy(out=xtT[:], in_=pst[:])
                    nc.tensor.matmul(
                        out=pso[:], lhsT=xtT[:],
                        rhs=(w1_t[:] if h == 0 else w2_t[:]),
                        start=(h == 0), stop=(h == 1),
                    )
                v = workp.tile([P, F_OUT], FP32, tag="fo")
                nc.vector.tensor_tensor(
                    out=v[:], in0=pso[:], in1=brep_t[:], op=add
                )
                r_ = workp.tile([P, F_OUT], FP32, tag="fo2")
                nc.scalar.activation(r_[:], v[:], Relu)
                nc.sync.dma_start(out_d[b_ * P:(b_ + 1) * P, :], r_[:])

            Copy = mybir.ActivationFunctionType.Copy
            cast_flip = 0
            for k in range(1, K_CHEB):
                acc = {}          # (ph, group) -> psum tile

                def ps_sl(b_, phx, k=k, acc=acc):
                    g = b_ // 8
                    return acc[(phx, g)][:, (b_ % 8) * F_IN:(b_ % 8 + 1) * F_IN]

                def close_block(b_, phx, k=k):
                    if phx == 0:
                        if k == 1:
                            nc.vector.tensor_scalar(
                                out=esl(b_), in0=ps_sl(b_, 0),
                                scalar1=0.5, scalar2=None, op0=mult,
                            )
                        else:
                            nc.vector.tensor_tensor(
                                out=esl(b_), in0=ps_sl(b_, 0),
                                in1=ysl(b_, k - 2), op=sub,
                            )
                        return
                    if k == 1:
                        nc.vector.scalar_tensor_tensor(
                            out=ysl(b_, 1), in0=ps_sl(b_, 1),
                            scalar=0.5, in1=esl(b_),
                            op0=mult, op1=add,
                        )
                    else:
                        nc.vector.tensor_tensor(
                            out=ysl(b_, k), in0=ps_sl(b_, 1),
                            in1=esl(b_), op=add,
                        )
                    if k < K_CHEB - 1:
                        if b_ < NB_E:
                            nc.sync.dma_start(
                                agE_in[k][b_ * P:(b_ + 1) * P, :],
                                ysl(b_, k),
                            )
                        else:
                            bl = b_ - NB_E
                            nc.sync.dma_start(
                                agL_in[k][bl * P:(bl + 1) * P, :],
                                ysl(b_, k),
                            )
                        if b_ == NB_E - 1:
                            nc.gpsimd.collective_compute(
                                "AllGather", mybir.AluOpType.bypass,
                                replica_groups=rg,
                                ins=[agE_in[k][:].opt()],
                                outs=[tabE[k][:].opt()],
                            )
                        if b_ == NB - 1:
                            nc.gpsimd.collective_compute(
                                "AllGather", mybir.AluOpType.bypass,
                                replica_groups=rg,
                                ins=[agL_in[k][:].opt()],
                                outs=[tabL[k][:].opt()],
                            )
                    else:
                        final_block(b_)

                for (cs, ce) in chunks:
                    phx = tiles_seq[cs][0]
                    ntl = ce - cs
                    if phx == 0:
                        table = tabE0 if k == 1 else tabE[k - 1]
                    else:
                        table = tabL0 if k == 1 else tabL[k - 1]
                    ct = chunkp.tile([P, CH, F_IN], FP32, tag="ct",
                                     name=f"ct{k}_{cs}", bufs=4)
                    nc.gpsimd.dma_gather(
                        ct[:, 0:ntl, :], table[:],
                        idx_t[:, cs * 8:ce * 8],
                        ntl * P, ntl * P, F_IN,
                    )
                    mt = mchp.tile([P, CH * P], BF16, tag="mt",
                                   name=f"mt{k}_{cs}", bufs=4)
                    nc.sync.dma_start(
                        mt[:, 0:ntl * P], m_d[:, cs * P:ce * P]
                    )
                    ctb = chunkp.tile([P, CH, F_IN], BF16, tag="ctb",
                                      name=f"ctb{k}_{cs}", bufs=4)
                    if cast_flip == 0:
                        nc.vector.tensor_copy(out=ctb[:, 0:ntl, :],
                                              in_=ct[:, 0:ntl, :])
                    else:
                        nc.scalar.activation(ctb[:, 0:ntl, :],
                                             ct[:, 0:ntl, :], Copy)
                    cast_flip ^= 1
                    for j in range(cs, ce):
                        _, b_, last = tiles_seq[j]
                        g = b_ // 8
                        if (phx, g) not in acc:
                            acc[(phx, g)] = psp.tile(
                                [P, 512], FP32, tag="acc",
                                name=f"acc{k}_{phx}_{g}", bufs=3,
                            )
                            nc.tensor.matmul(
                                out=acc[(phx, g)][:],
                                lhsT=zero_t[:, 0:P], rhs=zero_t[:],
                                start=True, stop=False,
                                skip_group_check=True,
                            )
                        nc.tensor.matmul(
                            out=ps_sl(b_, phx),
                            lhsT=mt[:, (j - cs) * P:(j - cs + 1) * P],
                            rhs=ctb[:, j - cs, :],
                            start=False, stop=last,
                            skip_group_check=True,
                        )
                        if last:
                            close_block(b_, phx)

    nc.compile()
    return nc


# ---------------------------------------------------------------------------
# entry point
# ---------------------------------------------------------------------------
_CACHE = {}


def _run(signal, src, dst, lambda_max, W, b, trace=False):
    cfg, in_maps = preprocess(signal, src, dst, lambda_max, W, b)
    key = (cfg["total_tiles"], cfg["tiles_seq"])
    if key not in _CACHE:
        _CACHE[key] = build_program(cfg)
    nc = _CACHE[key]
    res = run_bass_kernel_spmd(
        nc, in_maps, core_ids=list(range(N_CORES)), trace=trace
    )
    n_shard = cfg["n_shard"]
    outs = []
    for c in range(N_CORES):
        o = res.results[c]["out"]                      # [6272, 256]
        outs.append(o[:n_shard])
    full = np.concatenate(outs, axis=0)[:cfg["n_nodes"]]
    return full, res


def kernel(signal, src, dst, lambda_max, W, b):
    signal = np.asarray(signal, np.float32)
    src = np.asarray(src, np.int32)
    dst = np.asarray(dst, np.int32)
    lambda_max = np.asarray(lambda_max, np.float32)
    W = np.asarray(W, np.float32)
    b = np.asarray(b, np.float32)
    out, _ = _run(signal, src, dst, lambda_max, W, b, trace=False)
    return out


# revision 22
# speedup vs baseline: 1.1086x; 1.0618x over previous
"""ChebConv (K=4) GNN kernel for 8 Trainium2 NeuronCores — v4.

Strategy (1D node partition, pull-mode, matmul-scatter):
  - Nodes sharded 8 ways (6250/core, padded to 6272 = 49 blocks of 128).
  - Block-major local indexing: local node r -> block bk = r//128, row
    p = r%128. Blocks split into Early (bk < 25, 3200 rows/shard) and
    Late (bk >= 25, 3072 rows/shard) halves, each with its own gather
    table (25600 / 24576 rows, both < 2**15 so int16 indices cover them
    without windowing).
  - States y_k = d^{-1/2} X_k.  Per iteration k = 1..3, two phases:
    phase E gathers the contributions of Early-half sources, phase L of
    Late-half sources.  Per (block, phase): dma_gather (SWDGE) the edge
    slots, then per 128-slot tile one matmul ps += M.T @ gathered
    (M host-precomputed bf16 scatter matrices streamed from DRAM; M
    folds -2*re * cnt * ds2[dst]).  Phase E result is saved to SBUF
    (pre-combined with the recurrence term), phase L completes:
      y1 = 0.5*(psE + psL);  y_k = (psE + psL) - y_{k-2}  (lambda_max=2).
  - Trailing pad slots use idx = -1: the gather ucode trims them at
    runtime, so each core pays Q7 descriptor-generation time only for
    its real edges (the compiled call size covers the worst core).
  - Publication: as soon as all Early blocks of iteration k close, an
    AllGather publishes the Early table for iteration k+1 (overlapping
    the rest of iteration k); the Late AllGather at iteration end
    overlaps the next iteration's Early phase.  Iteration 1 gathers
    directly from host-uploaded y0 tables (no staging copy).
  - Final per block: xt = idsq * [y0|y1|y2|y3]; 2 PE transposes ->
    out = relu(xtT.T @ W + b) -> DMA out (overlaps iteration 3).
"""

import math
import sys

import numpy as np

sys.path.insert(0, "/opt/trn_rl_repo")

import concourse.bacc as bacc  # noqa: E402
import concourse.bass as bass  # noqa: E402
import concourse.mybir as mybir  # noqa: E402
import concourse.tile as tile  # noqa: E402
from concourse.bass_utils import run_bass_kernel_spmd  # noqa: E402

P = 128
N_CORES = 8
F_IN = 64
K_CHEB = 4
F_OUT = 256
FP32 = mybir.dt.float32
BF16 = mybir.dt.bfloat16
I16 = mybir.dt.int16

NB = 49            # blocks per shard
NB_E = 25          # early blocks
NB_L = NB - NB_E   # late blocks
ROWS_E = NB_E * P  # 3200
ROWS_L = NB_L * P  # 3072
CH = 8             # max tiles per gather call (1024 idx ucode limit)


# ---------------------------------------------------------------------------
# host-side graph preprocessing
# ---------------------------------------------------------------------------
def preprocess(signal, src, dst, lambda_max, W, b):
    import ml_dtypes

    n_nodes = signal.shape[0]
    n_shard = (n_nodes + N_CORES - 1) // N_CORES          # 6250
    assert NB * P >= n_shard and (NB - 1) * P < n_shard

    deg = np.bincount(dst, minlength=n_nodes).astype(np.float64)
    degc = np.maximum(deg, 1.0)
    dsqrt = (degc ** -0.5).astype(np.float32)
    ds2 = (1.0 / degc).astype(np.float32)
    idsq = (degc ** 0.5).astype(np.float32)

    re = 2.0 / float(np.asarray(lambda_max).reshape(-1)[0])
    c1 = re - 1.0
    assert abs(c1) < 1e-12, "general lambda_max not wired (needs c1 terms)"
    scale1 = np.float32(-2.0 * re)

    # dedup (dst, src) -> cnt
    key = dst.astype(np.int64) * n_nodes + src.astype(np.int64)
    ukey, cnt = np.unique(key, return_counts=True)
    udst = (ukey // n_nodes).astype(np.int64)
    usrc = (ukey % n_nodes).astype(np.int64)

    owner = udst // n_shard
    rloc = udst - owner * n_shard
    bk = rloc // P
    dr = rloc - bk * P

    sc = usrc // n_shard
    rs = usrc - sc * n_shard
    ph = (rs >= ROWS_E).astype(np.int64)
    trow = np.where(ph == 0, sc * ROWS_E + rs, sc * ROWS_L + (rs - ROWS_E))

    w_edge = (scale1 * cnt.astype(np.float32) * ds2[udst]).astype(np.float32)

    # slot = (owner, bk, ph, src); dedup via unique on composite key
    okey = (((owner * NB + bk) * 2 + ph) * n_nodes + usrc)
    slots_key, slot_of_edge = np.unique(okey, return_inverse=True)
    n_slots = len(slots_key)
    s_owner = slots_key // (2 * n_nodes * NB)
    rem = slots_key % (2 * n_nodes * NB)
    s_bk = rem // (2 * n_nodes)
    rem = rem % (2 * n_nodes)
    s_ph = rem // n_nodes
    s_src = rem % n_nodes
    s_sc = s_src // n_shard
    s_rs = s_src - s_sc * n_shard
    s_trow = np.where(s_ph == 0, s_sc * ROWS_E + s_rs,
                      s_sc * ROWS_L + (s_rs - ROWS_E))

    # per (owner, bk, ph) group: count + start (slots_key sorted => groups
    # contiguous in (owner, bk, ph) order)
    gid = (s_owner * NB + s_bk) * 2 + s_ph
    n_groups = N_CORES * NB * 2
    gcnt = np.bincount(gid, minlength=n_groups).reshape(N_CORES, NB, 2)
    gstart = np.zeros(n_groups + 1, dtype=np.int64)
    np.cumsum(gcnt.reshape(-1), out=gstart[1:])
    slot_local = np.arange(n_slots) - gstart[gid]

    # compiled tiles per (bk, ph): cover worst core
    gmax = gcnt.max(axis=0)                               # [NB, 2]
    tiles_bp = np.maximum(1, np.ceil(gmax / P).astype(np.int64))

    # stream order: ph-major, then block; tiles_seq = (ph, bk, last_of_block)
    tile_off = np.zeros((NB, 2), dtype=np.int64)
    tiles_seq = []
    toff = 0
    for phx in range(2):
        for b_ in range(NB):
            t = int(tiles_bp[b_, phx])
            tile_off[b_, phx] = toff
            for j in range(t):
                tiles_seq.append((phx, b_, j == t - 1))
            toff += t
    total_tiles = toff

    # per-core idx + M blob
    in_maps = []
    for c in range(N_CORES):
        idx16 = np.zeros((total_tiles, P), dtype=np.int16)
        mflat = np.zeros(P * total_tiles * P, dtype=np.float32)
        mask = s_owner == c
        sl = slot_local[mask]
        tr = s_trow[mask]
        gb = s_bk[mask]
        gp = s_ph[mask]
        gtile = tile_off[gb, gp] + sl // P
        idx16[gtile, sl % P] = tr.astype(np.int16)
        # M[slot%P (partition), gtile*P + dr (free)]
        emask = owner == c
        es = slot_of_edge[emask]
        # es indexes into global slot arrays; recompute per-edge placement
        e_sl = slot_local[es]
        e_tile = tile_off[s_bk[es], s_ph[es]] + e_sl // P
        e_part = e_sl % P
        e_dr = dr[emask]
        mflat[(e_part * total_tiles + e_tile) * P + e_dr] = w_edge[emask]
        mblob = mflat.reshape(P, total_tiles * P)
        wrap = idx16.reshape(-1, 16).T.copy()             # [16, tt*8]
        in_maps.append({
            "idx": np.tile(wrap, (8, 1)),                 # [128, tt*8]
            "mblob": mblob.astype(ml_dtypes.bfloat16),
        })

    # per-core aux arrays (block-major local layout: node r = bk*128+p)
    def cols_of(vec, fill):
        out = np.full((N_CORES, NB * P), fill, dtype=np.float32)
        out[:, :n_shard] = vec.reshape(N_CORES, n_shard)
        return out.reshape(N_CORES, NB, P).transpose(0, 2, 1).copy()

    idsq_cols = cols_of(idsq, 1.0)

    y0 = np.asarray(signal, np.float32) * dsqrt[:, None]
    y0_pad = np.zeros((N_CORES, NB * P, F_IN), dtype=np.float32)
    y0_pad[:, :n_shard] = y0.reshape(N_CORES, n_shard, F_IN)
    # y0 tables (global upload, identical on all cores)
    t0E = y0_pad[:, :ROWS_E].reshape(N_CORES * ROWS_E, F_IN).copy()
    t0L = y0_pad[:, ROWS_E:].reshape(N_CORES * ROWS_L, F_IN).copy()
    # y0 in SBUF layout [128, nb*64] (partition p, col bk*64+f)
    y0_sb = y0_pad.reshape(N_CORES, NB, P, F_IN).transpose(0, 2, 1, 3) \
        .reshape(N_CORES, P, NB * F_IN).copy()

    w_in = np.asarray(W, np.float32)
    b_rep = np.broadcast_to(np.asarray(b, np.float32), (P, F_OUT)).copy()
    ident = np.eye(P, dtype=np.float32)

    for c in range(N_CORES):
        in_maps[c].update({
            "t0E": t0E, "t0L": t0L,
            "y0sb": y0_sb[c].copy(),
            "idsq": idsq_cols[c],
            "w_in": w_in, "b_rep": b_rep, "ident": ident,
        })

    cfg = dict(
        n_nodes=n_nodes, n_shard=n_shard,
        total_tiles=total_tiles, tiles_seq=tuple(tiles_seq),
    )
    return cfg, in_maps


# ---------------------------------------------------------------------------
# Bass program
# ---------------------------------------------------------------------------
def build_program(cfg):
    total_tiles = cfg["total_tiles"]
    tiles_seq = cfg["tiles_seq"]

    # uniform CH-tile chunks, not crossing phase boundaries
    bounds = [0]
    for i in range(1, total_tiles):
        if tiles_seq[i][0] != tiles_seq[i - 1][0]:
            bounds.append(i)
    bounds.append(total_tiles)
    chunks = []
    for bi in range(len(bounds) - 1):
        s = bounds[bi]
        while s < bounds[bi + 1]:
            e = min(s + CH, bounds[bi + 1])
            chunks.append((s, e))
            s = e

    nc = bacc.Bacc(
        "TRN2", target_bir_lowering=False, debug=False,
        enable_asserts=False, num_devices=N_CORES,
    )

    t0E_d = nc.dram_tensor("t0E", [N_CORES * ROWS_E, F_IN], FP32,
                           kind="ExternalInput")
    t0L_d = nc.dram_tensor("t0L", [N_CORES * ROWS_L, F_IN], FP32,
                           kind="ExternalInput")
    idx_d = nc.dram_tensor("idx", [P, total_tiles * 8], I16,
                           kind="ExternalInput")
    m_d = nc.dram_tensor("mblob", [P, total_tiles * P], BF16,
                         kind="ExternalInput")
    y0sb_d = nc.dram_tensor("y0sb", [P, NB * F_IN], FP32,
                            kind="ExternalInput")
    idsq_d = nc.dram_tensor("idsq", [P, NB], FP32, kind="ExternalInput")
    w_d = nc.dram_tensor("w_in", [2 * P, F_OUT], FP32, kind="ExternalInput")
    brep_d = nc.dram_tensor("b_rep", [P, F_OUT], FP32, kind="ExternalInput")
    ident_d = nc.dram_tensor("ident", [P, P], FP32, kind="ExternalInput")
    out_d = nc.dram_tensor("out", [NB * P, F_OUT], FP32,
                           kind="ExternalOutput")

    rg = [list(range(N_CORES))]
    mult = mybir.AluOpType.mult
    add = mybir.AluOpType.add
    sub = mybir.AluOpType.subtract
    Relu = mybir.ActivationFunctionType.Relu

    with tile.TileContext(nc) as tc:
        with (
            tc.tile_pool(name="const", bufs=1) as constp,
            tc.tile_pool(name="state", bufs=1) as statep,
            tc.tile_pool(name="chunk", bufs=3) as chunkp,
            tc.tile_pool(name="mchunk", bufs=3) as mchp,
            tc.tile_pool(name="work", bufs=4) as workp,
            tc.tile_pool(name="psA", bufs=3, space="PSUM") as psp,
            tc.tile_pool(name="psT", bufs=2, space="PSUM") as pstp,
            tc.tile_pool(name="psO", bufs=1, space="PSUM") as psop,
            tc.tile_pool(name="dram", bufs=2, space="DRAM") as dramp,
        ):
            # staged y0 tables (SWDGE gather needs compile-time addresses,
            # so inputs must be copied into internal DRAM first)
            tabE0 = dramp.tile([N_CORES * ROWS_E, F_IN], FP32,
                               tag="tabE0", name="tabE0", bufs=1)
            nc.sync.dma_start(tabE0[:], t0E_d[:])
            tabL0 = dramp.tile([N_CORES * ROWS_L, F_IN], FP32,
                               tag="tabL0", name="tabL0", bufs=1)
            nc.sync.dma_start(tabL0[:], t0L_d[:])

            # publish buffers + gather tables for iterations 2, 3
            agE_in = {}
            agL_in = {}
            tabE = {}
            tabL = {}
            for k in (1, 2):
                agE_in[k] = dramp.tile([ROWS_E, F_IN], FP32, tag="agE",
                                       name=f"agE{k}", bufs=2)
                agL_in[k] = dramp.tile([ROWS_L, F_IN], FP32, tag="agL",
                                       name=f"agL{k}", bufs=2)
                tabE[k] = dramp.tile([N_CORES * ROWS_E, F_IN], FP32,
                                     tag="tabE", name=f"tabE{k}", bufs=2)
                tabL[k] = dramp.tile([N_CORES * ROWS_L, F_IN], FP32,
                                     tag="tabL", name=f"tabL{k}", bufs=2)
            # ---- constants
            idx_t = constp.tile([P, total_tiles * 8], I16, tag="idx")
            nc.sync.dma_start(idx_t[:], idx_d[:])
            idsq_t = constp.tile([P, NB], FP32, tag="idsq")
            nc.sync.dma_start(idsq_t[:], idsq_d[:])
            w1_t = constp.tile([P, F_OUT], FP32, tag="w1")
            nc.sync.dma_start(w1_t[:], w_d[0:P, :])
            w2_t = constp.tile([P, F_OUT], FP32, tag="w2")
            nc.sync.dma_start(w2_t[:], w_d[P:2 * P, :])
            brep_t = constp.tile([P, F_OUT], FP32, tag="brep")
            nc.sync.dma_start(brep_t[:], brep_d[:])
            ident_t = constp.tile([P, P], FP32, tag="ident")
            nc.sync.dma_start(ident_t[:], ident_d[:])
            zero_t = constp.tile([P, 512], FP32, tag="zero")
            nc.vector.memset(zero_t[:], 0.0)

            # ---- states
            ybuf = statep.tile([P, NB * 4 * F_IN], FP32, tag="ybuf")
            for b_ in range(NB):
                nc.sync.dma_start(
                    ybuf[:, b_ * 256:b_ * 256 + F_IN],
                    y0sb_d[:, b_ * F_IN:(b_ + 1) * F_IN],
                )
            esb = statep.tile([P, NB * F_IN], FP32, tag="esb")

            def ysl(b_, k):
                o = b_ * 256 + k * F_IN
                return ybuf[:, o:o + F_IN]

            def esl(b_):
                return esb[:, b_ * F_IN:(b_ + 1) * F_IN]

            # prime the gather-destination buffers (pads are skipped by the
            # ucode and would otherwise expose uninitialized SBUF -> NaN*0)
            for r_ in range(4):
                ctp = chunkp.tile([P, CH, F_IN], FP32, tag="ct",
                                  name=f"prime{r_}", bufs=4)
                nc.vector.memset(ctp[:], 0.0)

            def final_block(b_):
                xt = workp.tile([P, 4 * F_IN], FP32, tag="xt")
                nc.vector.tensor_scalar(
                    out=xt[:], in0=ybuf[:, b_ * 256:(b_ + 1) * 256],
                    scalar1=idsq_t[:, b_:b_ + 1], scalar2=None, op0=mult,
                )
                pso = psop.tile([P, F_OUT], FP32, tag="po")
                for h in range(2):
                    pst = pstp.tile([P, P], FP32, tag="tp")
                    nc.tensor.transpose(
                        pst[:], xt[:, h * P:(h + 1) * P], ident_t[:]
                    )
                    xtT = workp.tile([P, P], FP32, tag="xtT")
                    nc.vector.tensor_copy(out=xtT[:], in_=pst[:])
                    nc.tensor.matmul(
                        out=pso[:], lhsT=xtT[:],
                        rhs=(w1_t[:] if h == 0 else w2_t[:]),
                        start=(h == 0), stop=(h == 1),
                    )
                v = workp.tile([P, F_OUT], FP32, tag="fo")
                nc.vector.tensor_tensor(
                    out=v[:], in0=pso[:], in1=brep_t[:], op=add
                )
                r_ = workp.tile([P, F_OUT], FP32, tag="fo2")
                nc.scalar.activation(r_[:], v[:], Relu)
                nc.sync.dma_start(out_d[b_ * P:(b_ + 1) * P, :], r_[:])

            Copy = mybir.ActivationFunctionType.Copy
            cast_flip = 0
            for k in range(1, K_CHEB):
                acc = {}          # (ph, group) -> psum tile

                def ps_sl(b_, phx, k=k, acc=acc):
                    g = b_ // 8
                    return acc[(phx, g)][:, (b_ % 8) * F_IN:(b_ % 8 + 1) * F_IN]

                def close_block(b_, phx, k=k):
                    if phx == 0:
                        if k == 1:
                            nc.vector.tensor_scalar(
                                out=esl(b_), in0=ps_sl(b_, 0),
                                scalar1=0.5, scalar2=None, op0=mult,
                            )
                        else:
                            nc.vector.tensor_tensor(
                                out=esl(b_), in0=ps_sl(b_, 0),
                                in1=ysl(b_, k - 2), op=sub,
                            )
                        return
                    if k == 1:
                        nc.vector.scalar_tensor_tensor(
                            out=ysl(b_, 1), in0=ps_sl(b_, 1),
                            scalar=0.5, in1=esl(b_),
                            op0=mult, op1=add,
                        )
                    else:
                        nc.vector.tensor_tensor(
                            out=ysl(b_, k), in0=ps_sl(b_, 1),
                            in1=esl(b_), op=add,
                        )
                    if k < K_CHEB - 1:
                        if b_ < NB_E:
                            nc.sync.dma_start(
                                agE_in[k][b_ * P:(b_ + 1) * P, :],
                                ysl(b_, k),
                            )
                        else:
                            bl = b_ - NB_E
                            nc.sync.dma_start(
                                agL_in[k][bl * P:(bl + 1) * P, :],
                                ysl(b_, k),
                            )
                        if b_ == NB_E - 1:
                            nc.gpsimd.collective_compute(
                                "AllGather", mybir.AluOpType.bypass,
                                replica_groups=rg,
                                ins=[agE_in[k][:].opt()],
                                outs=[tabE[k][:].opt()],
                            )
                        if b_ == NB - 1:
                            nc.gpsimd.collective_compute(
                                "AllGather", mybir.AluOpType.bypass,
                                replica_groups=rg,
                                ins=[agL_in[k][:].opt()],
                                outs=[tabL[k][:].opt()],
                            )
                    else:
                        final_block(b_)

                for (cs, ce) in chunks:
                    phx = tiles_seq[cs][0]
                    ntl = ce - cs
                    if phx == 0:
                        table = tabE0 if k == 1 else tabE[k - 1]
                    else:
                        table = tabL0 if k == 1 else tabL[k - 1]
                    ct = chunkp.tile([P, CH, F_IN], FP32, tag="ct",
                                     name=f"ct{k}_{cs}", bufs=4)
                    nc.gpsimd.dma_gather(
                        ct[:, 0:ntl, :], table[:],
                        idx_t[:, cs * 8:ce * 8],
                        ntl * P, ntl * P, F_IN,
                    )
                    mt = mchp.tile([P, CH * P], BF16, tag="mt",
                                   name=f"mt{k}_{cs}", bufs=4)
                    nc.sync.dma_start(
                        mt[:, 0:ntl * P], m_d[:, cs * P:ce * P]
                    )
                    ctb = chunkp.tile([P, CH, F_IN], BF16, tag="ctb",
                                      name=f"ctb{k}_{cs}", bufs=4)
                    if cast_flip == 0:
                        nc.vector.tensor_copy(out=ctb[:, 0:ntl, :],
                                              in_=ct[:, 0:ntl, :])
                    else:
                        nc.scalar.activation(ctb[:, 0:ntl, :],
                                             ct[:, 0:ntl, :], Copy)
                    cast_flip ^= 1
                    for j in range(cs, ce):
                        _, b_, last = tiles_seq[j]
                        g = b_ // 8
                        if (phx, g) not in acc:
                            acc[(phx, g)] = psp.tile(
                                [P, 512], FP32, tag="acc",
                                name=f"acc{k}_{phx}_{g}", bufs=3,
                            )
                            nc.tensor.matmul(
                                out=acc[(phx, g)][:],
                                lhsT=zero_t[:, 0:P], rhs=zero_t[:],
                                start=True, stop=False,
                                skip_group_check=True,
                            )
                        nc.tensor.matmul(
                            out=ps_sl(b_, phx),
                            lhsT=mt[:, (j - cs) * P:(j - cs + 1) * P],
                            rhs=ctb[:, j - cs, :],
                            start=False, stop=last,
                            skip_group_check=True,
                        )
                        if last:
                            close_block(b_, phx)

    nc.compile()
    return nc


# ---------------------------------------------------------------------------
# entry point
# ---------------------------------------------------------------------------
_CACHE = {}


def _run(signal, src, dst, lambda_max, W, b, trace=False):
    cfg, in_maps = preprocess(signal, src, dst, lambda_max, W, b)
    key = (cfg["total_tiles"], cfg["tiles_seq"])
    if key not in _CACHE:
        _CACHE[key] = build_program(cfg)
    nc = _CACHE[key]
    res = run_bass_kernel_spmd(
        nc, in_maps, core_ids=list(range(N_CORES)), trace=trace
    )
    n_shard = cfg["n_shard"]
    outs = []
    for c in range(N_CORES):
        o = res.results[c]["out"]                      # [6272, 256]
        outs.append(o[:n_shard])
    full = np.concatenate(outs, axis=0)[:cfg["n_nodes"]]
    return full, res


def kernel(signal, src, dst, lambda_max, W, b):
    signal = np.asarray(signal, np.float32)
    src = np.asarray(src, np.int32)
    dst = np.asarray(dst, np.int32)
    lambda_max = np.asarray(lambda_max, np.float32)
    W = np.asarray(W, np.float32)
    b = np.asarray(b, np.float32)
    out, _ = _run(signal, src, dst, lambda_max, W, b, trace=False)
    return out
